# revision 1
# baseline (speedup 1.0000x reference)
"""Trainium2 Bass kernel for the analytic ellipsoid renderer (nn_AnalyticRenderer).

reference math:
  out[v,u,w] = sum_n where(disc>0, |S rn| * sqrt(disc), 0)
which algebraically reduces (ray-normalizations cancel; S @ Sinv = I) to
  out[v,u,w] = sum_n sqrt(relu(F_nv(u,w))) / q_nv(u,w)
    q  = |Sinv K pix|^2                      (quadratic bilinear form in u,w)
    F  = 4 * |K pix|^2 * ((Cn.g)^2 - ctil*q) (quartic bilinear form)
with pix=[u,w,1], K = inv(P[:, :3,:3]), and per-(n,v) constants from P,M,S.

Device strategy (8 NeuronCores, SPMD; one graph, per-core coefficient data):
  - image split into 32 row-tiles (122 rows x 976 cols); 4 tiles per core
  - per tile, up to S[j] (ellipsoid) sub-items; schedule shape shared SPMD
  - per sub-item: PE evaluates F and q via K=20/K=12 matmuls against
    hi/lo-split bf16 per-row-coefficient weights and w-power features
    (per-item basis center; ill-conditioned items use their epipole column);
    ACT computes s = Sqrt(F) (NaN where F<0); a custom fused DVE op computes
    z = relu(s) * recip_1NR(q) (relu kills the NaN mask); an fp16 identity
    matmul accumulates z into the PSUM accumulator (the sum over ellipsoids).
  - per tile: ACT copies the PSUM accumulator to SBUF, DMA to DRAM out.
"""
import sys
import os

sys.path.insert(0, "/opt/trn_rl_repo")

import numpy as np
import ml_dtypes
from math import comb

import concourse.bass as bass
import concourse.bacc as bacc
import concourse.tile as tile
import concourse.mybir as mybir
from concourse.bass_utils import run_bass_kernel_spmd

V, N, U, W = 4, 8, 976, 976
TROWS = 122
NTILES = U // TROWS
WCENTER = 487.5
RECIP_C0 = -0.23549792
RECIP_C1 = 2.0017324
ILL_THRESH = 1.5e-3
f32 = mybir.dt.float32
f16 = mybir.dt.float16
bf16 = mybir.dt.bfloat16

# --------------------------------------------------------------------------
# custom DVE op: out = relu(Src1) * recip_1nr(Src0)
# --------------------------------------------------------------------------
from concourse.dve_spec import Spec, Bin, AluOp, Src0, Src1, relu as dve_relu, C0, C1, lower
from concourse.dve_uop import DveOpSpec
import concourse.dve_ops as dve_ops
from concourse.dve_ops import DveOp


def _ref_relu_mul_recip1nr(in0, in1, c0, c1, c2):
    not_x = (~in0.view(np.int32)).view(np.float32)
    y0 = not_x * c0
    y1 = y0 * (c1 - in0 * y0)
    s = np.maximum(np.nan_to_num(in1.astype(np.float32), nan=0.0), 0.0)
    return s * y1


def _register_zop():
    name = "RELU_MUL_RECIP1NR_ANT"
    if name in dve_ops._SUB_OPCODE_FOR_NAME:
        for op in dve_ops.OPS:
            if op.name == name:
                return op
    _not_x = Bin(AluOp.BITWISE_NOT, Src0, Src0)
    _y0 = _not_x * C0
    _y1 = _y0 * (C1 - Src0 * _y0)
    spec = Spec(body=dve_relu(Src1) * _y1, reference=_ref_relu_mul_recip1nr)
    row = max(dve_ops._SUB_OPCODE_FOR_NAME.values()) + 1
    shas = {}
    for ver in ("v3", "v4"):
        try:
            uops = lower(spec, ver=ver)
            shas[ver] = DveOpSpec(name=name, opcode=row, uops=uops, rd1_en=True).sha(ver)
        except Exception:
            pass
    op = DveOp(name, spec, subdim=False, uops_sha=shas)
    dve_ops.OPS.append(op)
    dve_ops.CUSTOM_DVE_SPECS[name] = spec
    dve_ops._SUB_OPCODE_FOR_NAME[name] = row
    return op


ZOP = _register_zop()

# --------------------------------------------------------------------------
# host precompute (see derivation in module docstring)
# --------------------------------------------------------------------------


def _geometry(P, M, S):
    P64, M64, S64 = P.astype(np.float64), M.astype(np.float64), S.astype(np.float64)
    K = np.linalg.inv(P64[:, :3, :3])
    C = -np.einsum('vij,vj->vi', K, P64[:, :3, 3])
    Sinv = np.linalg.inv(S64)
    Q = np.einsum('nij,vjk->nvik', Sinv, K)
    Cn = np.einsum('nij,vnj->vni', Sinv, C[:, None, :] - M64[None, :, :])
    a_vec = np.einsum('nvji,vnj->nvi', Q, Cn)
    ctil = np.einsum('vni,vni->vn', Cn, Cn) - 1.0
    G = np.einsum('nvji,nvjk->nvik', Q, Q)
    KtK = np.einsum('vji,vjk->vik', K, K)
    return a_vec, ctil, G, KtK


def _quad_to_mat(B):
    B = 0.5 * (B + B.T)
    Mq = np.zeros((3, 3))
    Mq[2, 0] = B[0, 0]; Mq[0, 2] = B[1, 1]; Mq[0, 0] = B[2, 2]
    Mq[1, 1] = 2 * B[0, 1]; Mq[1, 0] = 2 * B[0, 2]; Mq[0, 1] = 2 * B[1, 2]
    return Mq


def _bilinear_forms(P, M, S):
    a_vec, ctil, G, KtK = _geometry(P, M, S)
    Fm = np.zeros((V, N, 5, 5)); qm = np.zeros((V, N, 3, 3))
    for v in range(V):
        rrm = _quad_to_mat(KtK[v])
        for n in range(N):
            qm[v, n] = _quad_to_mat(G[n, v])
            a = a_vec[n, v]
            dotm = np.zeros((3, 3))
            dotm[2, 0] = a[0] ** 2; dotm[0, 2] = a[1] ** 2; dotm[0, 0] = a[2] ** 2
            dotm[1, 1] = 2 * a[0] * a[1]; dotm[1, 0] = 2 * a[0] * a[2]
            dotm[0, 1] = 2 * a[1] * a[2]
            Dtm = dotm - ctil[v, n] * qm[v, n]
            Fm5 = np.zeros((5, 5))
            for i in range(3):
                for j in range(3):
                    Fm5[i:i + 3, j:j + 3] += 4.0 * rrm[i, j] * Dtm
            Fm[v, n] = Fm5
    return Fm, qm


def _shift_T(deg, c):
    T = np.zeros((deg, deg))
    for j in range(deg):
        for p in range(j + 1):
            T[j, p] = comb(j, p) * c ** (j - p)
    return T


def _split_hi_lo(x):
    x32 = np.asarray(x, dtype=np.float32)
    hi = x32.astype(ml_dtypes.bfloat16)
    lo = (x32 - hi.astype(np.float32)).astype(ml_dtypes.bfloat16)
    return hi, lo


def _feat_block(c, deg):
    wp = np.arange(W, dtype=np.float64) - c
    pows = np.stack([wp ** p for p in range(deg)], axis=0)
    hi, lo = _split_hi_lo(pows)
    return np.concatenate([hi, lo, hi, lo], axis=0)


def _pack_w(coeffs_T):
    hi, lo = _split_hi_lo(coeffs_T)
    return np.concatenate([hi, hi, lo, lo], axis=0)


def _prepare(P, M, S_in):
    Fm, qm = _bilinear_forms(P, M, S_in)
    u = np.arange(U, dtype=np.float64)
    ub5 = np.stack([u ** k for k in range(5)], axis=1)
    Fc = np.einsum('up,vnpj,jq->vnuq', ub5, Fm, _shift_T(5, WCENTER))
    qc = np.einsum('up,vnpj,jq->vnuq', ub5[:, :3], qm, _shift_T(3, WCENTER))

    wp = np.arange(W, dtype=np.float64) - WCENTER
    wb5 = np.stack([wp ** k for k in range(5)], axis=1)
    wb3 = wb5[:, :3]

    act = np.zeros((V, N, NTILES), dtype=bool)
    fmax = np.zeros((V, N, NTILES))
    qmin = np.zeros((V, N, NTILES))
    qterms = np.zeros((V, N, NTILES))
    for v in range(V):
        for n in range(N):
            Fg = (Fc[v, n] @ wb5.T).reshape(NTILES, TROWS, W)
            qg = (qc[v, n] @ wb3.T).reshape(NTILES, TROWS, W)
            act[v, n] = (Fg > 0).any(axis=(1, 2))
            fmax[v, n] = Fg.max(axis=(1, 2))
            qmin[v, n] = qg.min(axis=(1, 2))
            qt = (np.abs(qc[v, n]) * np.array([1.0, 488.0, 488.0 ** 2])).sum(axis=1)
            qterms[v, n] = qt.reshape(NTILES, TROWS).max(axis=1)
    ill = act & (qmin < qterms * ILL_THRESH)

    # per-half activity: active[v,n,t,h] over w-halves of each row-tile
    act_h = np.zeros((V, N, NTILES, 2), dtype=bool)
    fmax_h = np.zeros((V, N, NTILES, 2))
    for v in range(V):
        for n in range(N):
            Fg = (Fc[v, n] @ wb5.T).reshape(NTILES, TROWS, 2, 488)
            act_h[v, n] = (Fg > 0).any(axis=(1, 3))
            fmax_h[v, n] = Fg.max(axis=(1, 3))

    items = []
    for v in range(V):
        for t in range(NTILES):
            ns_h = [[n for n in range(N) if act_h[v, n, t, h]] for h in range(2)]
            items.append(((v, t), ns_h, len(ns_h[0]) + len(ns_h[1])))
    items.sort(key=lambda x: -x[2])
    buckets = [[] for _ in range(8)]
    for i, it in enumerate(items):
        buckets[i % 8].append(it)
    Sh = [[max(max(len(b[j][1][h]) for b in buckets), 1) for h in range(2)]
          for j in range(4)]
    flat = [Sh[j][h] for j in range(4) for h in range(2)]
    HH = sum(flat)
    hoffs = np.cumsum([0] + flat[:-1]).reshape(4, 2)

    # matmul operands need 32-aligned base partitions: 4 half-items per block
    nb = (HH + 3) // 4
    HW = 488
    wfs = np.zeros((8, 128, nb * TROWS), dtype=ml_dtypes.bfloat16)
    wqs = np.zeros((8, 128, nb * TROWS), dtype=ml_dtypes.bfloat16)
    fbankF = np.zeros((8, 128, nb * HW), dtype=ml_dtypes.bfloat16)
    fbankq = np.zeros((8, 128, nb * HW), dtype=ml_dtypes.bfloat16)
    slotmap = [[None] * 4 for _ in range(8)]

    featF_c = _feat_block(WCENTER, 5)
    featq_c = _feat_block(WCENTER, 3)

    for c in range(8):
        for j in range(4):
            (v, t), ns_h, _ = buckets[c][j]
            slotmap[c][j] = (v, t)
            rows = np.s_[t * TROWS:(t + 1) * TROWS]
            u_abs = np.arange(t * TROWS, (t + 1) * TROWS, dtype=np.float64)
            ub5t = np.stack([u_abs ** k2 for k2 in range(5)], axis=1)
            for h in range(2):
                for s in range(Sh[j][h]):
                    idx = int(hoffs[j][h]) + s
                    pP, bB = 32 * (idx % 4), idx // 4
                    slW = np.s_[pP:pP + 20, bB * TROWS:(bB + 1) * TROWS]
                    slq = np.s_[pP:pP + 12, bB * TROWS:(bB + 1) * TROWS]
                    slFw = np.s_[pP:pP + 20, bB * HW:(bB + 1) * HW]
                    slqw = np.s_[pP:pP + 12, bB * HW:(bB + 1) * HW]
                    if s < len(ns_h[h]):
                        n = ns_h[h][s]
                        if ill[v, n, t]:
                            c2 = qc[v, n, rows, 2]; c1 = qc[v, n, rows, 1]
                            w0 = -c1 / (2 * c2)
                            m = qc[v, n, rows, 0] - c1 ** 2 / (4 * c2)
                            ustar = int(np.argmin(m))
                            cw = WCENTER + w0[ustar]
                            Fcc = np.einsum('up,pj,jq->uq', ub5t, Fm[v, n], _shift_T(5, cw))
                            qcc = np.einsum('up,pj,jq->uq', ub5t[:, :3], qm[v, n], _shift_T(3, cw))
                            fF = _feat_block(cw, 5); fq = _feat_block(cw, 3)
                        else:
                            Fcc = Fc[v, n, rows]; qcc = qc[v, n, rows]
                            fF = featF_c; fq = featq_c
                        fmx = max(float(np.sqrt(max(fmax_h[v, n, t, h], 1e-30))), 1e-30)
                        k = max(0.0, np.ceil(np.log2(fmx) - 12.0))
                        wfs[c][slW] = _pack_w((Fcc * 4.0 ** -k).T)
                        wqs[c][slq] = _pack_w((qcc * 2.0 ** -k).T)
                        fbankF[c][slFw] = fF[:, h * HW:(h + 1) * HW]
                        fbankq[c][slqw] = fq[:, h * HW:(h + 1) * HW]
                    else:
                        wqs[c, pP, bB * TROWS:(bB + 1) * TROWS] = 1.0
                        fbankq[c, pP, bB * HW:(bB + 1) * HW] = 1.0
    return dict(S=Sh, SS=HH, soffs=hoffs, wfs=wfs, wqs=wqs,
                fbankF=fbankF, fbankq=fbankq, slotmap=slotmap, nb=nb)


# --------------------------------------------------------------------------
# bass graph
# --------------------------------------------------------------------------


def _build_nc(Sh, hoffs, HH, reps=1):
    nb = (HH + 3) // 4
    HW = 488
    nc = bacc.Bacc(None, target_bir_lowering=False, debug=False)
    d_wfs = nc.declare_dram_parameter("wfs", [128, nb * TROWS], bf16, isOutput=False)
    d_wqs = nc.declare_dram_parameter("wqs", [128, nb * TROWS], bf16, isOutput=False)
    d_fbF = nc.declare_dram_parameter("fbF", [128, nb * HW], bf16, isOutput=False)
    d_fbq = nc.declare_dram_parameter("fbq", [128, nb * HW], bf16, isOutput=False)
    d_id = nc.declare_dram_parameter("ident", [128, 128], f16, isOutput=False)
    d_out = nc.declare_dram_parameter("out", [4, TROWS, W], f16, isOutput=True)

    with tile.TileContext(nc) as tc:
        with (
            tc.tile_pool(name="consts", bufs=1) as consts,
            tc.tile_pool(name="sz", bufs=8) as szp,
            tc.tile_pool(name="zp", bufs=10) as zpool,
            tc.tile_pool(name="op", bufs=3) as opool,
            tc.tile_pool(name="pF", bufs=3, space="PSUM") as pFp,
            tc.tile_pool(name="pq", bufs=3, space="PSUM") as pqp,
            tc.tile_pool(name="pacc", bufs=2, space="PSUM") as paccp,
        ):
            # weights first (small, needed by item 0), then feature banks in
            # 3-block chunks as separate tiles (per-tile DMA dependencies)
            t_wfs = consts.tile([128, nb * TROWS], bf16)
            t_wqs = consts.tile([128, nb * TROWS], bf16)
            t_id = consts.tile([128, 128], f16)
            nc.sync.dma_start(t_wfs[:], d_wfs[:])
            nc.scalar.dma_start(t_wqs[:], d_wqs[:])
            nc.scalar.dma_start(t_id[:], d_id[:])
            CHB = 3  # blocks per chunk
            nch = (nb + CHB - 1) // CHB
            fbF_t, fbq_t = [], []
            for k in range(nch):
                blks = min(CHB, nb - k * CHB)
                tF = consts.tile([128, blks * HW], bf16, tag=f"fbF{k}")
                tq = consts.tile([128, blks * HW], bf16, tag=f"fbq{k}")
                c0f = k * CHB * HW
                nc.sync.dma_start(tF[:], d_fbF[:, c0f:c0f + blks * HW])
                nc.scalar.dma_start(tq[:], d_fbq[:, c0f:c0f + blks * HW])
                fbF_t.append(tF)
                fbq_t.append(tq)

            def _body(_iv=None):
                ohi = 0
                for j in range(4):
                    o_big = opool.tile([128, 976], f16, tag="o")
                    for h in range(2):
                        # phase 1: evals + sqrt + z for all sub-items (PE stays
                        # in tiled row-group mode; no full-array interleaving)
                        zs = []
                        for s in range(Sh[j][h]):
                            idx = int(hoffs[j][h]) + s
                            pP, bB = 32 * (idx % 4), idx // 4
                            Ft = pFp.tile([128, 512], f32, tag="F")
                            qt = pqp.tile([128, 512], f32, tag="q")
                            nc.tensor.matmul(
                                Ft[0:TROWS, 0:488],
                                t_wfs[pP:pP + 20, bB * TROWS:(bB + 1) * TROWS],
                                fbF_t[bB // 3][pP:pP + 20, (bB % 3) * HW:(bB % 3 + 1) * HW],
                                start=True, stop=True, tile_position=(pP, 0),
                            )
                            nc.tensor.matmul(
                                qt[0:TROWS, 0:488],
                                t_wqs[pP:pP + 12, bB * TROWS:(bB + 1) * TROWS],
                                fbq_t[bB // 3][pP:pP + 12, (bB % 3) * HW:(bB % 3 + 1) * HW],
                                start=True, stop=True, tile_position=(pP, 0),
                            )
                            s_t = szp.tile([128, 488], f16, tag="s")
                            nc.scalar.activation(
                                s_t[0:TROWS, :], Ft[0:TROWS, 0:488],
                                mybir.ActivationFunctionType.Sqrt,
                            )
                            z_t = zpool.tile([128, 488], f16, tag="z")
                            nc.vector._custom_dve(
                                ZOP, out=z_t[0:TROWS, :], in0=qt[0:TROWS, 0:488],
                                in1=s_t[0:TROWS, :], s0=RECIP_C0, s1=RECIP_C1,
                            )
                            zs.append(z_t)
                        # phase 2: back-to-back identity accumulates (one weight
                        # set, no row-group mode switches between them).
                        # Contract rows 0:TROWS only — rows 122..127 of z are
                        # uninitialized SBUF and 0*NaN would poison columns.
                        acc = paccp.tile([128, 512], f32, tag="acc")
                        for s, z_t in enumerate(zs):
                            nc.tensor.matmul(
                                acc[:, 0:488], t_id[0:TROWS, :], z_t[0:TROWS, :],
                                start=(s == 0), stop=(s == len(zs) - 1),
                            )
                        # evacuate each half into one full-width fp16 tile;
                        # one striped 3-queue DMA per slot (single-queue HWDGE
                        # bandwidth is the bottleneck under 8-core load)
                        if h == 0:
                            nc.scalar.copy(o_big[0:TROWS, 0:488], acc[0:TROWS, 0:488])
                        else:
                            nc.vector.tensor_copy(o_big[0:TROWS, 488:976], acc[0:TROWS, 0:488])
                            qs = [nc.sync, nc.scalar, nc.gpsimd]
                            bounds = [0, 40, 80, TROWS]
                            for i in range(3):
                                p0, p1 = bounds[i], bounds[i + 1]
                                qs[i].dma_start(d_out[j, p0:p1, :], o_big[p0:p1, :])
                        ohi += 1
            if reps == 1:
                _body()
            else:
                hints = (mybir.EngineType.PE, mybir.EngineType.Activation,
                         mybir.EngineType.DVE, mybir.EngineType.SP,
                         mybir.EngineType.Pool)
                with tc.For_i(0, reps, 1, hint_engines=hints) as _iv:
                    _body(_iv)
    nc.compile()
    return nc


_CACHE = {}


def kernel(P, M, S):
    P = np.ascontiguousarray(np.asarray(P, dtype=np.float32))
    M = np.ascontiguousarray(np.asarray(M, dtype=np.float32))
    S = np.ascontiguousarray(np.asarray(S, dtype=np.float32))
    prep = _prepare(P, M, S)
    Ssch, soffs, SS = prep["S"], prep["soffs"], prep["SS"]

    key = tuple(x for row in Ssch for x in row)
    if key not in _CACHE:
        _CACHE[key] = _build_nc(Ssch, soffs, SS)
    nc = _CACHE[key]

    ident = np.eye(128, dtype=np.float16)
    in_maps = []
    for c in range(8):
        in_maps.append({
            "wfs": np.ascontiguousarray(prep["wfs"][c]).view(np.uint16),
            "wqs": np.ascontiguousarray(prep["wqs"][c]).view(np.uint16),
            "fbF": np.ascontiguousarray(prep["fbankF"][c]).view(np.uint16),
            "fbq": np.ascontiguousarray(prep["fbankq"][c]).view(np.uint16),
            "ident": ident,
        })
    res = run_bass_kernel_spmd(nc, in_maps, core_ids=list(range(8)))

    out = np.zeros((V, U, W), dtype=np.float32)
    for c in range(8):
        o = res.results[c]["out"]
        for j in range(4):
            v, t = prep["slotmap"][c][j]
            out[v, t * TROWS:(t + 1) * TROWS, :] = o[j].astype(np.float32)
    return out


if __name__ == "__main__":
    P = np.load(os.path.join(os.path.dirname(__file__), 'P.npy'))
    M = np.load(os.path.join(os.path.dirname(__file__), 'M.npy'))
    S = np.load(os.path.join(os.path.dirname(__file__), 'S.npy'))
    o = kernel(P=P, M=M, S=S)
    print("out", o.shape, o.dtype, float(np.linalg.norm(o)))



# revision 3
# speedup vs baseline: 1.4808x; 1.4808x over previous
"""Trainium2 Bass kernel for the analytic ellipsoid renderer (nn_AnalyticRenderer).

reference math:
  out[v,u,w] = sum_n where(disc>0, |S rn| * sqrt(disc), 0)
which algebraically reduces (ray-normalizations cancel; S @ Sinv = I) to
  out[v,u,w] = sum_n sqrt(relu(F_nv(u,w))) / q_nv(u,w)
    q  = |Sinv K pix|^2                      (quadratic bilinear form in u,w)
    F  = 4 * |K pix|^2 * ((Cn.g)^2 - ctil*q) (quartic bilinear form)
with pix=[u,w,1], K = inv(P[:, :3,:3]), and per-(n,v) constants from P,M,S.

Device strategy v2 (8 NeuronCores, SPMD; one graph, per-core data):
  - image split into 32 row-tiles (122 rows x 976 cols), LPT-balanced 4 per
    core; tiles split into 2 col-halves of 488; item = active (ellipsoid,
    tile, half); counts padded across cores per (slot, half).
  - per item PE evaluates F (K=20) and q (K=12) against ONE shared hi/lo
    bf16 feature bank (q rows are a prefix of the F rows; per-item feature
    blocks handle ill-conditioned epipole-centered bases).
  - per item ACT computes s = sqrt(F) (NaN where F<0); a custom DVE op
    computes z = relu(s)*recip_1NR(q).  The first item of each half writes
    its z straight into the PSUM accumulator; later items' z go to SBUF f16
    and are accumulated by f16 identity matmuls (start=False).
  - program order interleaves: evals of half H+1 are issued before the acc
    matmuls of half H, so PE never stalls on the sqrt->z latency chain.
  - per half: ACT copies the PSUM accumulator to SBUF f16; DMA to DRAM out
    col-split on the sync/scalar HWDGE queues.
"""
import sys
import os

sys.path.insert(0, "/opt/trn_rl_repo")

import numpy as np
import ml_dtypes
from math import comb

import concourse.bass as bass
import concourse.bacc as bacc
import concourse.tile as tile
import concourse.mybir as mybir
from concourse.bass_utils import run_bass_kernel_spmd

V, N, U, W = 4, 8, 976, 976
TROWS = 122
NTILES = U // TROWS
HW = 488
WCENTER = 487.5
RECIP_C0 = -0.23549792
RECIP_C1 = 2.0017324
ILL_THRESH = 1.5e-3
f32 = mybir.dt.float32
f16 = mybir.dt.float16
bf16 = mybir.dt.bfloat16

# --------------------------------------------------------------------------
# custom DVE op: out = relu(Src1) * recip_1nr(Src0)
# --------------------------------------------------------------------------
from concourse.dve_spec import Spec, Bin, AluOp, Src0, Src1, relu as dve_relu, C0, C1, lower
from concourse.dve_uop import DveOpSpec
import concourse.dve_ops as dve_ops
from concourse.dve_ops import DveOp


def _ref_relu_mul_recip1nr(in0, in1, c0, c1, c2):
    not_x = (~in0.view(np.int32)).view(np.float32)
    y0 = not_x * c0
    y1 = y0 * (c1 - in0 * y0)
    s = np.maximum(np.nan_to_num(in1.astype(np.float32), nan=0.0), 0.0)
    return s * y1


def _register_zop():
    name = "RELU_MUL_RECIP1NR_ANT"
    if name in dve_ops._SUB_OPCODE_FOR_NAME:
        for op in dve_ops.OPS:
            if op.name == name:
                return op
    _not_x = Bin(AluOp.BITWISE_NOT, Src0, Src0)
    _y0 = _not_x * C0
    _y1 = _y0 * (C1 - Src0 * _y0)
    spec = Spec(body=dve_relu(Src1) * _y1, reference=_ref_relu_mul_recip1nr)
    row = max(dve_ops._SUB_OPCODE_FOR_NAME.values()) + 1
    shas = {}
    for ver in ("v3", "v4"):
        try:
            uops = lower(spec, ver=ver)
            shas[ver] = DveOpSpec(name=name, opcode=row, uops=uops, rd1_en=True).sha(ver)
        except Exception:
            pass
    op = DveOp(name, spec, subdim=False, uops_sha=shas)
    dve_ops.OPS.append(op)
    dve_ops.CUSTOM_DVE_SPECS[name] = spec
    dve_ops._SUB_OPCODE_FOR_NAME[name] = row
    return op


ZOP = _register_zop()

# --------------------------------------------------------------------------
# host precompute
# --------------------------------------------------------------------------


def _geometry(P, M, S):
    P64, M64, S64 = P.astype(np.float64), M.astype(np.float64), S.astype(np.float64)
    K = np.linalg.inv(P64[:, :3, :3])
    C = -np.einsum('vij,vj->vi', K, P64[:, :3, 3])
    Sinv = np.linalg.inv(S64)
    Q = np.einsum('nij,vjk->nvik', Sinv, K)
    Cn = np.einsum('nij,vnj->vni', Sinv, C[:, None, :] - M64[None, :, :])
    a_vec = np.einsum('nvji,vnj->nvi', Q, Cn)
    ctil = np.einsum('vni,vni->vn', Cn, Cn) - 1.0
    G = np.einsum('nvji,nvjk->nvik', Q, Q)
    KtK = np.einsum('vji,vjk->vik', K, K)
    return a_vec, ctil, G, KtK


def _quad_to_mat(B):
    B = 0.5 * (B + B.T)
    Mq = np.zeros((3, 3))
    Mq[2, 0] = B[0, 0]; Mq[0, 2] = B[1, 1]; Mq[0, 0] = B[2, 2]
    Mq[1, 1] = 2 * B[0, 1]; Mq[1, 0] = 2 * B[0, 2]; Mq[0, 1] = 2 * B[1, 2]
    return Mq


def _bilinear_forms(P, M, S):
    a_vec, ctil, G, KtK = _geometry(P, M, S)
    Fm = np.zeros((V, N, 5, 5)); qm = np.zeros((V, N, 3, 3))
    for v in range(V):
        rrm = _quad_to_mat(KtK[v])
        for n in range(N):
            qm[v, n] = _quad_to_mat(G[n, v])
            a = a_vec[n, v]
            dotm = np.zeros((3, 3))
            dotm[2, 0] = a[0] ** 2; dotm[0, 2] = a[1] ** 2; dotm[0, 0] = a[2] ** 2
            dotm[1, 1] = 2 * a[0] * a[1]; dotm[1, 0] = 2 * a[0] * a[2]
            dotm[0, 1] = 2 * a[1] * a[2]
            Dtm = dotm - ctil[v, n] * qm[v, n]
            Fm5 = np.zeros((5, 5))
            for i in range(3):
                for j in range(3):
                    Fm5[i:i + 3, j:j + 3] += 4.0 * rrm[i, j] * Dtm
            Fm[v, n] = Fm5
    return Fm, qm


def _shift_T(deg, c):
    T = np.zeros((deg, deg))
    for j in range(deg):
        for p in range(j + 1):
            T[j, p] = comb(j, p) * c ** (j - p)
    return T


def _split_hi_lo(x):
    x32 = np.asarray(x, dtype=np.float32)
    hi = x32.astype(ml_dtypes.bfloat16)
    lo = (x32 - hi.astype(np.float32)).astype(ml_dtypes.bfloat16)
    return hi, lo


def _feat_rows(center, wlo, whi):
    """Feature rows (20) for abs cols [wlo, whi): q rows (0..11) are a prefix.
    Row order: [hi012, lo012, hi012, lo012, hi34, lo34, hi34, lo34]."""
    wp = np.arange(wlo, whi, dtype=np.float64) - center
    pows = np.stack([wp ** p for p in range(5)], axis=0)
    hi, lo = _split_hi_lo(pows)
    hi = hi.astype(np.float32); lo = lo.astype(np.float32)
    rows = np.concatenate([
        hi[0:3], lo[0:3], hi[0:3], lo[0:3],
        hi[3:5], lo[3:5], hi[3:5], lo[3:5],
    ], axis=0)
    return rows.astype(ml_dtypes.bfloat16)


def _pack_wF(coeffs):
    """coeffs (122, 5) -> (20, 122) weight rows matching _feat_rows."""
    hi, lo = _split_hi_lo(coeffs.T)
    hi = hi.astype(np.float32); lo = lo.astype(np.float32)
    rows = np.concatenate([
        hi[0:3], hi[0:3], lo[0:3], lo[0:3],
        hi[3:5], hi[3:5], lo[3:5], lo[3:5],
    ], axis=0)
    return rows.astype(ml_dtypes.bfloat16)


def _pack_wq(coeffs):
    """coeffs (122, 3) -> (12, 122) weight rows matching feature rows 0-11."""
    hi, lo = _split_hi_lo(coeffs.T)
    hi = hi.astype(np.float32); lo = lo.astype(np.float32)
    rows = np.concatenate([hi, hi, lo, lo], axis=0)
    return rows.astype(ml_dtypes.bfloat16)


def _prepare(P, M, S_in):
    Fm, qm = _bilinear_forms(P, M, S_in)
    u = np.arange(U, dtype=np.float64)
    ub5 = np.stack([u ** k for k in range(5)], axis=1)
    Fc = np.einsum('up,vnpj,jq->vnuq', ub5, Fm, _shift_T(5, WCENTER))
    qc = np.einsum('up,vnpj,jq->vnuq', ub5[:, :3], qm, _shift_T(3, WCENTER))

    wp = np.arange(W, dtype=np.float64) - WCENTER
    wb5 = np.stack([wp ** k for k in range(5)], axis=1)
    wb3 = wb5[:, :3]

    act_h = np.zeros((V, N, NTILES, 2), dtype=bool)
    fmax_h = np.zeros((V, N, NTILES, 2))
    ill = np.zeros((V, N, NTILES), dtype=bool)
    for v in range(V):
        for n in range(N):
            Fg = (Fc[v, n] @ wb5.T).reshape(NTILES, TROWS, 2, HW)
            qg = (qc[v, n] @ wb3.T).reshape(NTILES, TROWS, W)
            act_h[v, n] = (Fg > 0).any(axis=(1, 3))
            fmax_h[v, n] = Fg.max(axis=(1, 3))
            qt_ = (np.abs(qc[v, n]) * np.array([1.0, 488.0, 488.0 ** 2])).sum(axis=1)
            qterms = qt_.reshape(NTILES, TROWS).max(axis=1)
            for t in range(NTILES):
                if act_h[v, n, t].any() and qg.reshape(NTILES, TROWS, W)[t].min() < qterms[t] * ILL_THRESH:
                    ill[v, n, t] = True

    # schedule: tiles -> cores (LPT on active half-item count)
    tile_items = {}
    for v in range(V):
        for t in range(NTILES):
            its = [(h, n) for h in range(2) for n in range(N) if act_h[v, n, t, h]]
            tile_items[(v, t)] = its
    order = sorted(tile_items, key=lambda k: -len(tile_items[k]))
    load = [0] * 8
    core_tiles = [[] for _ in range(8)]
    for k in order:
        cands = [i for i in range(8) if len(core_tiles[i]) < 4]
        c = min(cands, key=lambda i: (load[i], len(core_tiles[i])))
        core_tiles[c].append(k)
        load[c] += len(tile_items[k])
    slotmap = [[None] * 4 for _ in range(8)]
    core_sched = [[[[] for _ in range(2)] for _ in range(4)] for _ in range(8)]
    for c in range(8):
        tl = sorted(core_tiles[c], key=lambda k: -len(tile_items[k]))
        while len(tl) < 4:
            tl.append(None)
        for j in range(4):
            slotmap[c][j] = tl[j]
            if tl[j] is None:
                continue
            for h in range(2):
                core_sched[c][j][h] = [n for (hh, n) in tile_items[tl[j]] if hh == h]

    nIt = [[max(len(core_sched[c][j][h]) for c in range(8)) for h in range(2)]
           for j in range(4)]
    n_items = sum(nIt[j][h] for j in range(4) for h in range(2))
    nblk = max(1, (n_items + 3) // 4)

    wfs = np.zeros((8, 128, nblk * TROWS), dtype=ml_dtypes.bfloat16)
    wqs = np.zeros((8, 128, nblk * TROWS), dtype=ml_dtypes.bfloat16)
    fbk = np.zeros((8, 128, nblk * HW), dtype=ml_dtypes.bfloat16)

    ui_all = np.arange(U, dtype=np.float64)
    item_band = {}
    idx = 0
    for j in range(4):
        for h in range(2):
            for s in range(nIt[j][h]):
                pP, blk = 32 * (idx % 4), idx // 4
                item_band[(j, h, s)] = (pP, blk)
                idx += 1
                wlo, whi = h * HW, (h + 1) * HW
                for c in range(8):
                    k = slotmap[c][j]
                    sched = core_sched[c][j][h] if k is not None else []
                    slW = np.s_[pP:pP + 20, blk * TROWS:(blk + 1) * TROWS]
                    slq = np.s_[pP:pP + 12, blk * TROWS:(blk + 1) * TROWS]
                    slf = np.s_[pP:pP + 20, blk * HW:(blk + 1) * HW]
                    if k is None or s >= len(sched):
                        wqs[c][pP, blk * TROWS:(blk + 1) * TROWS] = 1.0
                        fbk[c][pP, blk * HW:(blk + 1) * HW] = 1.0
                        continue
                    v, t = k
                    n = sched[s]
                    rows = np.s_[t * TROWS:(t + 1) * TROWS]
                    u_abs = ui_all[t * TROWS:(t + 1) * TROWS]
                    ub5t = np.stack([u_abs ** kk for kk in range(5)], axis=1)
                    if ill[v, n, t]:
                        c2 = qc[v, n, rows, 2]; c1 = qc[v, n, rows, 1]
                        with np.errstate(divide='ignore', invalid='ignore'):
                            wv = -c1 / (2 * c2)
                            mv = qc[v, n, rows, 0] - c1 ** 2 / (4 * c2)
                        mv = np.where(np.isfinite(mv), mv, np.inf)
                        ustar = int(np.argmin(mv))
                        cw = WCENTER + (wv[ustar] if np.isfinite(wv[ustar]) else 0.0)
                        Fcc = np.einsum('up,pj,jq->uq', ub5t, Fm[v, n], _shift_T(5, cw))
                        qcc = np.einsum('up,pj,jq->uq', ub5t[:, :3], qm[v, n], _shift_T(3, cw))
                        center = cw
                    else:
                        Fcc = Fc[v, n, rows]
                        qcc = qc[v, n, rows]
                        center = WCENTER
                    fmx = max(float(np.sqrt(max(fmax_h[v, n, t, h], 1e-30))), 1e-30)
                    kk = max(0.0, np.ceil(np.log2(fmx) - 12.0))
                    wfs[c][slW] = _pack_wF(Fcc * 4.0 ** -kk)
                    wqs[c][slq] = _pack_wq(qcc * 2.0 ** -kk)
                    fbk[c][slf] = _feat_rows(center, wlo, whi)
    return dict(nIt=nIt, nblk=nblk, wfs=wfs, wqs=wqs, fbk=fbk,
                slotmap=slotmap)


# --------------------------------------------------------------------------
# bass graph
# --------------------------------------------------------------------------

FB_CHUNK = 3  # feature-bank blocks per const DMA chunk


def _sched_key(prep):
    return (tuple(tuple(r) for r in prep["nIt"]), prep["nblk"])


def _build_nc(prep, reps=1):
    nIt, nblk = prep["nIt"], prep["nblk"]

    nc = bacc.Bacc(None, target_bir_lowering=False, debug=False)
    d_wfs = nc.declare_dram_parameter("wfs", [128, nblk * TROWS], bf16, isOutput=False)
    d_wqs = nc.declare_dram_parameter("wqs", [128, nblk * TROWS], bf16, isOutput=False)
    d_fbk = nc.declare_dram_parameter("fbk", [128, nblk * HW], bf16, isOutput=False)
    d_id = nc.declare_dram_parameter("ident", [128, 128], f16, isOutput=False)
    d_out = nc.declare_dram_parameter("out", [4, TROWS, W], f16, isOutput=True)

    item_band = {}
    idx = 0
    halves = []
    for j in range(4):
        for h in range(2):
            for s in range(nIt[j][h]):
                item_band[(j, h, s)] = (32 * (idx % 4), idx // 4)
                idx += 1
            halves.append((j, h))

    with tile.TileContext(nc) as tc:
        with (
            tc.tile_pool(name="consts", bufs=1) as consts,
            tc.tile_pool(name="sp", bufs=3) as spool,
            tc.tile_pool(name="zp", bufs=10) as zpool,
            tc.tile_pool(name="op", bufs=3) as opool,
            tc.tile_pool(name="pF", bufs=3, space="PSUM") as pFp,
            tc.tile_pool(name="pq", bufs=3, space="PSUM") as pqp,
            tc.tile_pool(name="pacc", bufs=2, space="PSUM") as paccp,
        ):
            t_wfs = consts.tile([128, nblk * TROWS], bf16)
            t_wqs = consts.tile([128, nblk * TROWS], bf16)
            t_id = consts.tile([128, 128], f16)
            nc.sync.dma_start(t_wfs[:], d_wfs[:])
            nc.scalar.dma_start(t_wqs[:], d_wqs[:])
            nc.scalar.dma_start(t_id[:], d_id[:])
            nch = (nblk + FB_CHUNK - 1) // FB_CHUNK
            fbk_t = []
            qs = [nc.sync, nc.scalar]
            for k in range(nch):
                pk = min(FB_CHUNK, nblk - k * FB_CHUNK)
                tF = consts.tile([128, pk * HW], bf16, tag=f"fbk{k}")
                c0 = k * FB_CHUNK * HW
                qs[k % 2].dma_start(tF[:], d_fbk[:, c0:c0 + pk * HW])
                fbk_t.append(tF)

            def fb_slice(blk, r0, nr):
                ch, off = blk // FB_CHUNK, (blk % FB_CHUNK) * HW
                return fbk_t[ch][r0:r0 + nr, off:off + HW]

            def _body(_iv=None):
                # pending: per half, list of (z_tile, ) for s>=1 items
                pend = {}
                accs = {}

                def emit_evals(jh):
                    j, h = jh
                    acc = paccp.tile([128, 512], f32, tag="acc")
                    accs[jh] = acc
                    pend[jh] = []
                    for s in range(nIt[j][h]):
                        pP, blk = item_band[(j, h, s)]
                        Fp = pFp.tile([128, 512], f32, tag="F")
                        qp = pqp.tile([128, 512], f32, tag="q")
                        nc.tensor.matmul(
                            Fp[0:TROWS, 0:HW],
                            t_wfs[pP:pP + 20, blk * TROWS:(blk + 1) * TROWS],
                            fb_slice(blk, pP, 20),
                            start=True, stop=True, tile_position=(pP, 0))
                        nc.tensor.matmul(
                            qp[0:TROWS, 0:HW],
                            t_wqs[pP:pP + 12, blk * TROWS:(blk + 1) * TROWS],
                            fb_slice(blk, pP, 12),
                            start=True, stop=True, tile_position=(pP, 0))
                        s_t = spool.tile([128, 512], f16, tag="s")
                        nc.scalar.activation(s_t[0:TROWS, 0:HW], Fp[0:TROWS, 0:HW],
                                             mybir.ActivationFunctionType.Sqrt)
                        if s == 0:
                            nc.vector._custom_dve(ZOP, out=acc[0:TROWS, 0:HW],
                                                  in0=qp[0:TROWS, 0:HW],
                                                  in1=s_t[0:TROWS, 0:HW],
                                                  s0=RECIP_C0, s1=RECIP_C1)
                        else:
                            z_t = zpool.tile([128, 512], f16, tag="z")
                            nc.vector._custom_dve(ZOP, out=z_t[0:TROWS, 0:HW],
                                                  in0=qp[0:TROWS, 0:HW],
                                                  in1=s_t[0:TROWS, 0:HW],
                                                  s0=RECIP_C0, s1=RECIP_C1)
                            pend[jh].append(z_t)

                def emit_accs(jh):
                    j, h = jh
                    acc = accs.pop(jh)
                    for z_t in pend.pop(jh):
                        nc.tensor.matmul(acc[0:TROWS, 0:HW], t_id[0:TROWS, 0:TROWS],
                                         z_t[0:TROWS, 0:HW],
                                         start=False, stop=True, skip_group_check=True)
                    o_t = opool.tile([128, 512], f16, tag="o")
                    nc.scalar.copy(o_t[0:TROWS, 0:HW], acc[0:TROWS, 0:HW])
                    qd = qs[(2 * j + h) % 2]
                    qd.dma_start(d_out[j, :, h * HW:(h + 1) * HW], o_t[0:TROWS, 0:HW])

                live = [jh for jh in halves if nIt[jh[0]][jh[1]] > 0]
                for i, jh in enumerate(live):
                    emit_evals(jh)
                    if i >= 1:
                        emit_accs(live[i - 1])
                if live:
                    emit_accs(live[-1])
                for (j, h) in halves:
                    if nIt[j][h] == 0:
                        o_t = opool.tile([128, 512], f16, tag="o")
                        nc.vector.memset(o_t[0:TROWS, 0:HW], 0.0)
                        qd = qs[(2 * j + h) % 2]
                        qd.dma_start(d_out[j, :, h * HW:(h + 1) * HW],
                                     o_t[0:TROWS, 0:HW])

            if reps == 1:
                _body()
            else:
                hints = (mybir.EngineType.PE, mybir.EngineType.Activation,
                         mybir.EngineType.DVE, mybir.EngineType.SP)
                with tc.For_i(0, reps, 1, hint_engines=hints) as _iv:
                    _body(_iv)
    nc.compile()
    return nc


_CACHE = {}


def _in_maps(prep):
    ident = np.eye(128, dtype=np.float16)
    maps = []
    for c in range(8):
        maps.append({
            "wfs": np.ascontiguousarray(prep["wfs"][c]).view(np.uint16),
            "wqs": np.ascontiguousarray(prep["wqs"][c]).view(np.uint16),
            "fbk": np.ascontiguousarray(prep["fbk"][c]).view(np.uint16),
            "ident": ident,
        })
    return maps


def kernel(P, M, S):
    P = np.ascontiguousarray(np.asarray(P, dtype=np.float32))
    M = np.ascontiguousarray(np.asarray(M, dtype=np.float32))
    S = np.ascontiguousarray(np.asarray(S, dtype=np.float32))
    prep = _prepare(P, M, S)

    key = _sched_key(prep)
    if key not in _CACHE:
        _CACHE[key] = _build_nc(prep)
    nc = _CACHE[key]

    res = run_bass_kernel_spmd(nc, _in_maps(prep), core_ids=list(range(8)))

    out = np.zeros((V, U, W), dtype=np.float32)
    for c in range(8):
        o = res.results[c]["out"]
        for j in range(4):
            k = prep["slotmap"][c][j]
            if k is None:
                continue
            v, t = k
            out[v, t * TROWS:(t + 1) * TROWS, :] = o[j].astype(np.float32)
    return out


if __name__ == "__main__":
    P = np.load(os.path.join(os.path.dirname(__file__), 'P.npy'))
    M = np.load(os.path.join(os.path.dirname(__file__), 'M.npy'))
    S = np.load(os.path.join(os.path.dirname(__file__), 'S.npy'))
    o = kernel(P=P, M=M, S=S)
    print("out", o.shape, o.dtype, float(np.linalg.norm(o)))


# revision 6
# speedup vs baseline: 1.7310x; 1.1690x over previous
"""Trainium2 Bass kernel for the analytic ellipsoid renderer (nn_AnalyticRenderer).

reference math:
  out[v,u,w] = sum_n where(disc>0, |S rn| * sqrt(disc), 0)
which algebraically reduces (ray-normalizations cancel; S @ Sinv = I) to
  out[v,u,w] = sum_n sqrt(relu(F_nv(u,w))) / q_nv(u,w)
    q  = |Sinv K pix|^2                      (quadratic bilinear form in u,w)
    F  = 4 * |K pix|^2 * ((Cn.g)^2 - ctil*q) (quartic bilinear form)
with pix=[u,w,1], K = inv(P[:, :3,:3]), and per-(n,v) constants from P,M,S.

Device strategy v2 (8 NeuronCores, SPMD; one graph, per-core data):
  - image split into 32 row-tiles (122 rows x 976 cols), LPT-balanced 4 per
    core; tiles split into 2 col-halves of 488; item = active (ellipsoid,
    tile, half); counts padded across cores per (slot, half).
  - per item PE evaluates F (K=20) and q (K=12) against ONE shared hi/lo
    bf16 feature bank (q rows are a prefix of the F rows; per-item feature
    blocks handle ill-conditioned epipole-centered bases).
  - per item ACT computes s = sqrt(F) (NaN where F<0); a custom DVE op
    computes z = relu(s)*recip_1NR(q).  The first item of each half writes
    its z straight into the PSUM accumulator; later items' z go to SBUF f16
    and are accumulated by f16 identity matmuls (start=False).
  - program order interleaves: evals of half H+1 are issued before the acc
    matmuls of half H, so PE never stalls on the sqrt->z latency chain.
  - per half: ACT copies the PSUM accumulator to SBUF f16; DMA to DRAM out
    col-split on the sync/scalar HWDGE queues.
"""
import sys
import os

sys.path.insert(0, "/opt/trn_rl_repo")

import numpy as np
import ml_dtypes
from math import comb

import concourse.bass as bass
import concourse.bacc as bacc
import concourse.tile as tile
import concourse.mybir as mybir
from concourse.bass_utils import run_bass_kernel_spmd

V, N, U, W = 4, 8, 976, 976
TROWS = 122
NTILES = U // TROWS
HW = 488
WCENTER = 487.5
RECIP_C0 = -0.23549792
RECIP_C1 = 2.0017324
ILL_THRESH = 1.5e-3
f32 = mybir.dt.float32
f16 = mybir.dt.float16
bf16 = mybir.dt.bfloat16

# --------------------------------------------------------------------------
# custom DVE op: out = relu(Src1) * recip_1nr(Src0)
# --------------------------------------------------------------------------
from concourse.dve_spec import Spec, Bin, AluOp, Src0, Src1, relu as dve_relu, C0, C1, lower
from concourse.dve_uop import DveOpSpec
import concourse.dve_ops as dve_ops
from concourse.dve_ops import DveOp


def _ref_relu_mul_recip1nr(in0, in1, c0, c1, c2):
    not_x = (~in0.view(np.int32)).view(np.float32)
    y0 = not_x * c0
    y1 = y0 * (c1 - in0 * y0)
    s = np.maximum(np.nan_to_num(in1.astype(np.float32), nan=0.0), 0.0)
    return s * y1


def _register_zop():
    name = "RELU_MUL_RECIP1NR_ANT"
    if name in dve_ops._SUB_OPCODE_FOR_NAME:
        for op in dve_ops.OPS:
            if op.name == name:
                return op
    _not_x = Bin(AluOp.BITWISE_NOT, Src0, Src0)
    _y0 = _not_x * C0
    _y1 = _y0 * (C1 - Src0 * _y0)
    spec = Spec(body=dve_relu(Src1) * _y1, reference=_ref_relu_mul_recip1nr)
    row = max(dve_ops._SUB_OPCODE_FOR_NAME.values()) + 1
    shas = {}
    for ver in ("v3", "v4"):
        try:
            uops = lower(spec, ver=ver)
            shas[ver] = DveOpSpec(name=name, opcode=row, uops=uops, rd1_en=True).sha(ver)
        except Exception:
            pass
    op = DveOp(name, spec, subdim=False, uops_sha=shas)
    dve_ops.OPS.append(op)
    dve_ops.CUSTOM_DVE_SPECS[name] = spec
    dve_ops._SUB_OPCODE_FOR_NAME[name] = row
    return op


ZOP = _register_zop()

# --------------------------------------------------------------------------
# host precompute
# --------------------------------------------------------------------------


def _geometry(P, M, S):
    P64, M64, S64 = P.astype(np.float64), M.astype(np.float64), S.astype(np.float64)
    K = np.linalg.inv(P64[:, :3, :3])
    C = -np.einsum('vij,vj->vi', K, P64[:, :3, 3])
    Sinv = np.linalg.inv(S64)
    Q = np.einsum('nij,vjk->nvik', Sinv, K)
    Cn = np.einsum('nij,vnj->vni', Sinv, C[:, None, :] - M64[None, :, :])
    a_vec = np.einsum('nvji,vnj->nvi', Q, Cn)
    ctil = np.einsum('vni,vni->vn', Cn, Cn) - 1.0
    G = np.einsum('nvji,nvjk->nvik', Q, Q)
    KtK = np.einsum('vji,vjk->vik', K, K)
    return a_vec, ctil, G, KtK


def _quad_to_mat(B):
    B = 0.5 * (B + B.T)
    Mq = np.zeros((3, 3))
    Mq[2, 0] = B[0, 0]; Mq[0, 2] = B[1, 1]; Mq[0, 0] = B[2, 2]
    Mq[1, 1] = 2 * B[0, 1]; Mq[1, 0] = 2 * B[0, 2]; Mq[0, 1] = 2 * B[1, 2]
    return Mq


def _bilinear_forms(P, M, S):
    a_vec, ctil, G, KtK = _geometry(P, M, S)
    Fm = np.zeros((V, N, 5, 5)); qm = np.zeros((V, N, 3, 3))
    for v in range(V):
        rrm = _quad_to_mat(KtK[v])
        for n in range(N):
            qm[v, n] = _quad_to_mat(G[n, v])
            a = a_vec[n, v]
            dotm = np.zeros((3, 3))
            dotm[2, 0] = a[0] ** 2; dotm[0, 2] = a[1] ** 2; dotm[0, 0] = a[2] ** 2
            dotm[1, 1] = 2 * a[0] * a[1]; dotm[1, 0] = 2 * a[0] * a[2]
            dotm[0, 1] = 2 * a[1] * a[2]
            Dtm = dotm - ctil[v, n] * qm[v, n]
            Fm5 = np.zeros((5, 5))
            for i in range(3):
                for j in range(3):
                    Fm5[i:i + 3, j:j + 3] += 4.0 * rrm[i, j] * Dtm
            Fm[v, n] = Fm5
    return Fm, qm


def _shift_T(deg, c):
    T = np.zeros((deg, deg))
    for j in range(deg):
        for p in range(j + 1):
            T[j, p] = comb(j, p) * c ** (j - p)
    return T


def _split_hi_lo(x):
    x32 = np.asarray(x, dtype=np.float32)
    hi = x32.astype(ml_dtypes.bfloat16)
    lo = (x32 - hi.astype(np.float32)).astype(ml_dtypes.bfloat16)
    return hi, lo


def _feat_rows(center, wlo, whi):
    """Feature rows (20) for abs cols [wlo, whi): q rows (0..11) are a prefix.
    Row order: [hi012, lo012, hi012, lo012, hi34, lo34, hi34, lo34]."""
    wp = np.arange(wlo, whi, dtype=np.float64) - center
    pows = np.stack([wp ** p for p in range(5)], axis=0)
    hi, lo = _split_hi_lo(pows)
    hi = hi.astype(np.float32); lo = lo.astype(np.float32)
    rows = np.concatenate([
        hi[0:3], lo[0:3], hi[0:3], lo[0:3],
        hi[3:5], lo[3:5], hi[3:5], lo[3:5],
    ], axis=0)
    return rows.astype(ml_dtypes.bfloat16)


def _pack_wF(coeffs):
    """coeffs (122, 5) -> (20, 122) weight rows matching _feat_rows."""
    hi, lo = _split_hi_lo(coeffs.T)
    hi = hi.astype(np.float32); lo = lo.astype(np.float32)
    rows = np.concatenate([
        hi[0:3], hi[0:3], lo[0:3], lo[0:3],
        hi[3:5], hi[3:5], lo[3:5], lo[3:5],
    ], axis=0)
    return rows.astype(ml_dtypes.bfloat16)


def _pack_wq(coeffs):
    """coeffs (122, 3) -> (12, 122) weight rows matching feature rows 0-11."""
    hi, lo = _split_hi_lo(coeffs.T)
    hi = hi.astype(np.float32); lo = lo.astype(np.float32)
    rows = np.concatenate([hi, hi, lo, lo], axis=0)
    return rows.astype(ml_dtypes.bfloat16)


def _prepare(P, M, S_in):
    Fm, qm = _bilinear_forms(P, M, S_in)
    u = np.arange(U, dtype=np.float64)
    ub5 = np.stack([u ** k for k in range(5)], axis=1)
    Fc = np.einsum('up,vnpj,jq->vnuq', ub5, Fm, _shift_T(5, WCENTER))
    qc = np.einsum('up,vnpj,jq->vnuq', ub5[:, :3], qm, _shift_T(3, WCENTER))

    wp = np.arange(W, dtype=np.float64) - WCENTER
    wb5 = np.stack([wp ** k for k in range(5)], axis=1)
    wb3 = wb5[:, :3]

    act_h = np.zeros((V, N, NTILES, 2), dtype=bool)
    fmax_h = np.zeros((V, N, NTILES, 2))
    ill = np.zeros((V, N, NTILES), dtype=bool)
    for v in range(V):
        for n in range(N):
            Fg = (Fc[v, n] @ wb5.T).reshape(NTILES, TROWS, 2, HW)
            qg = (qc[v, n] @ wb3.T).reshape(NTILES, TROWS, W)
            act_h[v, n] = (Fg > 0).any(axis=(1, 3))
            fmax_h[v, n] = Fg.max(axis=(1, 3))
            qt_ = (np.abs(qc[v, n]) * np.array([1.0, 488.0, 488.0 ** 2])).sum(axis=1)
            qterms = qt_.reshape(NTILES, TROWS).max(axis=1)
            for t in range(NTILES):
                if act_h[v, n, t].any() and qg.reshape(NTILES, TROWS, W)[t].min() < qterms[t] * ILL_THRESH:
                    ill[v, n, t] = True

    # schedule: tiles -> cores (LPT on active half-item count)
    tile_items = {}
    for v in range(V):
        for t in range(NTILES):
            its = [(h, n) for h in range(2) for n in range(N) if act_h[v, n, t, h]]
            tile_items[(v, t)] = its
    order = sorted(tile_items, key=lambda k: -len(tile_items[k]))
    load = [0] * 8
    core_tiles = [[] for _ in range(8)]
    for k in order:
        cands = [i for i in range(8) if len(core_tiles[i]) < 4]
        c = min(cands, key=lambda i: (load[i], len(core_tiles[i])))
        core_tiles[c].append(k)
        load[c] += len(tile_items[k])
    slotmap = [[None] * 4 for _ in range(8)]
    core_sched = [[[[] for _ in range(2)] for _ in range(4)] for _ in range(8)]
    for c in range(8):
        tl = sorted(core_tiles[c], key=lambda k: -len(tile_items[k]))
        while len(tl) < 4:
            tl.append(None)
        for j in range(4):
            slotmap[c][j] = tl[j]
            if tl[j] is None:
                continue
            for h in range(2):
                core_sched[c][j][h] = [n for (hh, n) in tile_items[tl[j]] if hh == h]

    nIt = [[max(len(core_sched[c][j][h]) for c in range(8)) for h in range(2)]
           for j in range(4)]
    n_items = sum(nIt[j][h] for j in range(4) for h in range(2))
    nblk = max(1, (n_items + 3) // 4)

    wfs = np.zeros((8, 128, nblk * TROWS), dtype=ml_dtypes.bfloat16)
    wqs = np.zeros((8, 128, nblk * TROWS), dtype=ml_dtypes.bfloat16)
    fbk = np.zeros((8, 128, nblk * HW), dtype=ml_dtypes.bfloat16)

    ui_all = np.arange(U, dtype=np.float64)
    item_band = {}
    idx = 0
    for j in range(4):
        for h in range(2):
            for s in range(nIt[j][h]):
                pP, blk = 32 * (idx % 4), idx // 4
                item_band[(j, h, s)] = (pP, blk)
                idx += 1
                wlo, whi = h * HW, (h + 1) * HW
                for c in range(8):
                    k = slotmap[c][j]
                    sched = core_sched[c][j][h] if k is not None else []
                    slW = np.s_[pP:pP + 20, blk * TROWS:(blk + 1) * TROWS]
                    slq = np.s_[pP:pP + 12, blk * TROWS:(blk + 1) * TROWS]
                    slf = np.s_[pP:pP + 20, blk * HW:(blk + 1) * HW]
                    if k is None or s >= len(sched):
                        wqs[c][pP, blk * TROWS:(blk + 1) * TROWS] = 1.0
                        fbk[c][pP, blk * HW:(blk + 1) * HW] = 1.0
                        continue
                    v, t = k
                    n = sched[s]
                    rows = np.s_[t * TROWS:(t + 1) * TROWS]
                    u_abs = ui_all[t * TROWS:(t + 1) * TROWS]
                    ub5t = np.stack([u_abs ** kk for kk in range(5)], axis=1)
                    if ill[v, n, t]:
                        c2 = qc[v, n, rows, 2]; c1 = qc[v, n, rows, 1]
                        with np.errstate(divide='ignore', invalid='ignore'):
                            wv = -c1 / (2 * c2)
                            mv = qc[v, n, rows, 0] - c1 ** 2 / (4 * c2)
                        mv = np.where(np.isfinite(mv), mv, np.inf)
                        ustar = int(np.argmin(mv))
                        cw = WCENTER + (wv[ustar] if np.isfinite(wv[ustar]) else 0.0)
                        Fcc = np.einsum('up,pj,jq->uq', ub5t, Fm[v, n], _shift_T(5, cw))
                        qcc = np.einsum('up,pj,jq->uq', ub5t[:, :3], qm[v, n], _shift_T(3, cw))
                        center = cw
                    else:
                        Fcc = Fc[v, n, rows]
                        qcc = qc[v, n, rows]
                        center = WCENTER
                    fmx = max(float(np.sqrt(max(fmax_h[v, n, t, h], 1e-30))), 1e-30)
                    kk = max(0.0, np.ceil(np.log2(fmx) - 12.0))
                    wfs[c][slW] = _pack_wF(Fcc * 4.0 ** -kk)
                    wqs[c][slq] = _pack_wq(qcc * 2.0 ** -kk)
                    fbk[c][slf] = _feat_rows(center, wlo, whi)
    return dict(nIt=nIt, nblk=nblk, wfs=wfs, wqs=wqs, fbk=fbk,
                slotmap=slotmap)


# --------------------------------------------------------------------------
# bass graph
# --------------------------------------------------------------------------

FB_CHUNK = 3  # feature-bank blocks per const DMA chunk


def _sched_key(prep):
    return (tuple(tuple(r) for r in prep["nIt"]), prep["nblk"])


def _build_nc(prep, reps=1):
    nIt, nblk = prep["nIt"], prep["nblk"]

    nc = bacc.Bacc(None, target_bir_lowering=False, debug=False)
    d_wfs = nc.declare_dram_parameter("wfs", [128, nblk * TROWS], bf16, isOutput=False)
    d_wqs = nc.declare_dram_parameter("wqs", [128, nblk * TROWS], bf16, isOutput=False)
    d_fbk = nc.declare_dram_parameter("fbk", [128, nblk * HW], bf16, isOutput=False)
    d_id = nc.declare_dram_parameter("ident", [128, 128], f16, isOutput=False)
    d_out = nc.declare_dram_parameter("out", [4, TROWS, W], f16, isOutput=True)

    item_band = {}
    idx = 0
    halves = []
    for j in range(4):
        for h in range(2):
            for s in range(nIt[j][h]):
                item_band[(j, h, s)] = (32 * (idx % 4), idx // 4)
                idx += 1
            halves.append((j, h))

    with tile.TileContext(nc) as tc:
        with (
            tc.tile_pool(name="consts", bufs=1) as consts,
            tc.tile_pool(name="sp", bufs=3) as spool,
            tc.tile_pool(name="zp", bufs=10) as zpool,
            tc.tile_pool(name="op", bufs=3) as opool,
            tc.tile_pool(name="pF", bufs=3, space="PSUM") as pFp,
            tc.tile_pool(name="pq", bufs=3, space="PSUM") as pqp,
            tc.tile_pool(name="pacc", bufs=2, space="PSUM") as paccp,
        ):
            t_wfs = consts.tile([128, nblk * TROWS], bf16)
            t_wqs = consts.tile([128, nblk * TROWS], bf16)
            t_id = consts.tile([128, 128], f16)
            nc.sync.dma_start(t_wfs[:], d_wfs[:])
            nc.scalar.dma_start(t_wqs[:], d_wqs[:])
            nc.scalar.dma_start(t_id[:], d_id[:])
            nch = (nblk + FB_CHUNK - 1) // FB_CHUNK
            fbk_t = []
            qs = [nc.sync, nc.scalar]
            for k in range(nch):
                pk = min(FB_CHUNK, nblk - k * FB_CHUNK)
                tF = consts.tile([128, pk * HW], bf16, tag=f"fbk{k}")
                c0 = k * FB_CHUNK * HW
                qs[k % 2].dma_start(tF[:], d_fbk[:, c0:c0 + pk * HW])
                fbk_t.append(tF)

            def fb_slice(blk, r0, nr):
                ch, off = blk // FB_CHUNK, (blk % FB_CHUNK) * HW
                return fbk_t[ch][r0:r0 + nr, off:off + HW]


            def _body(_iv=None):
                # pending: per half, list of (z_tile, ) for s>=1 items
                pend = {}
                accs = {}

                def emit_evals(jh):
                    j, h = jh
                    acc = paccp.tile([128, 512], f32, tag="acc")
                    accs[jh] = acc
                    pend[jh] = []
                    for s in range(nIt[j][h]):
                        pP, blk = item_band[(j, h, s)]
                        Fp = pFp.tile([128, 512], f32, tag="F")
                        qp = pqp.tile([128, 512], f32, tag="q")
                        nc.tensor.matmul(
                            Fp[0:TROWS, 0:HW],
                            t_wfs[pP:pP + 20, blk * TROWS:(blk + 1) * TROWS],
                            fb_slice(blk, pP, 20),
                            start=True, stop=True, tile_position=(pP, 0))
                        nc.tensor.matmul(
                            qp[0:TROWS, 0:HW],
                            t_wqs[pP:pP + 12, blk * TROWS:(blk + 1) * TROWS],
                            fb_slice(blk, pP, 12),
                            start=True, stop=True, tile_position=(pP, 0))
                        s_t = spool.tile([128, 512], f16, tag="s")
                        nc.scalar.activation(s_t[0:TROWS, 0:HW], Fp[0:TROWS, 0:HW],
                                             mybir.ActivationFunctionType.Sqrt)
                        if s == 0:
                            nc.vector._custom_dve(ZOP, out=acc[0:TROWS, 0:HW],
                                                  in0=qp[0:TROWS, 0:HW],
                                                  in1=s_t[0:TROWS, 0:HW],
                                                  s0=RECIP_C0, s1=RECIP_C1)
                        else:
                            z_t = zpool.tile([128, 512], f16, tag="z")
                            nc.vector._custom_dve(ZOP, out=z_t[0:TROWS, 0:HW],
                                                  in0=qp[0:TROWS, 0:HW],
                                                  in1=s_t[0:TROWS, 0:HW],
                                                  s0=RECIP_C0, s1=RECIP_C1)
                            pend[jh].append(z_t)

                def emit_accs(jh):
                    j, h = jh
                    acc = accs.pop(jh)
                    for z_t in pend.pop(jh):
                        nc.tensor.matmul(acc[0:TROWS, 0:HW], t_id[0:TROWS, 0:TROWS],
                                         z_t[0:TROWS, 0:HW],
                                         start=False, stop=True, skip_group_check=True)
                    o_t = opool.tile([128, 512], f16, tag="o")
                    # split PSUM->SBUF copies between ACT and DVE (5/3)
                    if (2 * j + h) % 8 in (1, 4, 6):
                        nc.vector.tensor_copy(o_t[0:TROWS, 0:HW], acc[0:TROWS, 0:HW])
                    else:
                        nc.scalar.copy(o_t[0:TROWS, 0:HW], acc[0:TROWS, 0:HW])
                    qd = qs[(2 * j + h) % 2]
                    qd.dma_start(d_out[j, :, h * HW:(h + 1) * HW], o_t[0:TROWS, 0:HW])

                live = [jh for jh in halves if nIt[jh[0]][jh[1]] > 0]
                for i, jh in enumerate(live):
                    emit_evals(jh)
                    if i >= 1:
                        emit_accs(live[i - 1])
                if live:
                    emit_accs(live[-1])
                for (j, h) in halves:
                    if nIt[j][h] == 0:
                        o_t = opool.tile([128, 512], f16, tag="o")
                        nc.vector.memset(o_t[0:TROWS, 0:HW], 0.0)
                        qd = qs[(2 * j + h) % 2]
                        qd.dma_start(d_out[j, :, h * HW:(h + 1) * HW],
                                     o_t[0:TROWS, 0:HW])

            if reps == 1:
                _body()
            else:
                hints = (mybir.EngineType.PE, mybir.EngineType.Activation,
                         mybir.EngineType.DVE, mybir.EngineType.SP)
                with tc.For_i(0, reps, 1, hint_engines=hints) as _iv:
                    _body(_iv)
    nc.compile()
    return nc


_CACHE = {}


def _in_maps(prep):
    ident = np.eye(128, dtype=np.float16)
    maps = []
    for c in range(8):
        maps.append({
            "wfs": np.ascontiguousarray(prep["wfs"][c]).view(np.uint16),
            "wqs": np.ascontiguousarray(prep["wqs"][c]).view(np.uint16),
            "fbk": np.ascontiguousarray(prep["fbk"][c]).view(np.uint16),
            "ident": ident,
        })
    return maps


def kernel(P, M, S):
    P = np.ascontiguousarray(np.asarray(P, dtype=np.float32))
    M = np.ascontiguousarray(np.asarray(M, dtype=np.float32))
    S = np.ascontiguousarray(np.asarray(S, dtype=np.float32))
    prep = _prepare(P, M, S)

    key = _sched_key(prep)
    if key not in _CACHE:
        _CACHE[key] = _build_nc(prep)
    nc = _CACHE[key]

    res = run_bass_kernel_spmd(nc, _in_maps(prep), core_ids=list(range(8)))

    out = np.zeros((V, U, W), dtype=np.float32)
    for c in range(8):
        o = res.results[c]["out"]
        for j in range(4):
            k = prep["slotmap"][c][j]
            if k is None:
                continue
            v, t = k
            out[v, t * TROWS:(t + 1) * TROWS, :] = o[j].astype(np.float32)
    return out


if __name__ == "__main__":
    P = np.load(os.path.join(os.path.dirname(__file__), 'P.npy'))
    M = np.load(os.path.join(os.path.dirname(__file__), 'M.npy'))
    S = np.load(os.path.join(os.path.dirname(__file__), 'S.npy'))
    o = kernel(P=P, M=M, S=S)
    print("out", o.shape, o.dtype, float(np.linalg.norm(o)))
